# revision 11
# baseline (speedup 1.0000x reference)
"""Trainium2 Bass kernel for the GRUCell-variant problem.

  z = sigmoid(x@Wiz + h@Uhz + bz)
  r = sigmoid(x@Wir + h@Uhr + br)
  n = sigmoid(x@Win + (r*h)@Uhn + bn)
  out = (1-z)*h + z*n

Full shapes: x,h [8192,1024]; W*,U* [1024,1024]; b* [1024].
Sharding: data-parallel over batch across 8 NeuronCores (1024 rows each);
weights replicated; no collectives.

v3 design (fp16 compute, fp32 PSUM accumulate, zero device transposes):
  - Host pre-transposes x,h to feature-major [D, B_CORE] fp16 and biases
    to [128,8] fp32; weights are cast to fp16 in natural layout. The
    device output is feature-major [D, B_CORE] fp16; the host transposes
    back and upcasts. All layout shuffling is host-side numpy - the
    device does exclusively matmul + sigmoid + elementwise.
  - Everything SBUF-resident: xT,hT,rh + 6 weight matrices (fp16 halves
    the footprint so it all fits).
  - Matmuls are 768 x [128d x 128h stationary] @ [128d x 512b moving]
    fp16 (1 cyc/row): ~164us PE floor. Pairs of matmuls (batch half 0/1)
    share a stationary tile back-to-back.
  - R phase per-o weight chunks let the PE start ~1.5us in; ZN phase
    z-matmuls precede n-matmuls so rh (needs R complete) is never waited
    on.
"""

import sys

if "/opt/trn_rl_repo" not in sys.path:
    sys.path.insert(0, "/opt/trn_rl_repo")

import numpy as np

P = 128
B_FULL = 8192
D = 1024  # d_in == d_h == 1024
N_CORES = 8
B_CORE = B_FULL // N_CORES  # 1024
NS = D // P  # 8 strips of 128 along any 1024 dim
BH = 512  # moving free-dim per matmul (one PSUM bank in fp32)
NBH = B_CORE // BH  # 2 batch halves

_NC_CACHE = {}


def _build_bass():
    import concourse.mybir as mybir
    import concourse.tile as tile
    from concourse import bacc

    F16 = mybir.dt.float16
    F32 = mybir.dt.float32
    SIG = mybir.ActivationFunctionType.Sigmoid

    nc = bacc.Bacc(None, target_bir_lowering=False)

    # x,h arrive pre-transposed to feature-major [D, B] fp16
    xt = nc.dram_tensor("xt", [D, B_CORE], F16, kind="ExternalInput")
    ht = nc.dram_tensor("ht", [D, B_CORE], F16, kind="ExternalInput")
    wts = {
        name: nc.dram_tensor(name, [D, D], F16, kind="ExternalInput")
        for name in ("Wiz", "Uhz", "Wir", "Uhr", "Win", "Uhn")
    }
    bts = {
        name: nc.dram_tensor(name, [P, NS], F32, kind="ExternalInput")
        for name in ("bzt", "brt", "bnt")
    }
    # feature-major output; host transposes back
    out = nc.dram_tensor("out", [D, B_CORE], F16, kind="ExternalOutput")

    with tile.TileContext(nc) as tc:
        with (
            tc.tile_pool(name="big", bufs=1) as big,
            tc.tile_pool(name="dp", bufs=4) as dp,
            tc.tile_pool(name="gt", bufs=8) as gt,
            tc.tile_pool(name="ps", bufs=8, space="PSUM") as psp,
        ):
            # Persistent feature-major activations: [p, o, b] = val[o*128+p, b]
            xT = big.tile([P, NS, B_CORE], F16, tag="xT")
            hT = big.tile([P, NS, B_CORE], F16, tag="hT")
            rh = big.tile([P, NS, B_CORE], F16, tag="rh")
            # Weights natural layout: [p, o, n] = W[o*128+p, n]
            wsb = {}
            for name in ("Wir", "Uhr", "Wiz", "Uhz", "Win", "Uhn"):
                wsb[name] = big.tile(
                    [P, NS, D], F16, tag=f"w_{name}", name=f"w_{name}"
                )
            bias = {}
            for name in ("bzt", "brt", "bnt"):
                bt = big.tile([P, NS], F32, tag=name)
                bias[name] = bt

            # ---- inputs on SP queue: x chunks then h chunks (R-bh0
            # consumption order), then Wiz/Uhz for the ZN phase.
            for src, dst in ((xt, xT), (ht, hT)):
                for o in range(NS):
                    nc.sync.dma_start(
                        out=dst[:, o, :], in_=src.ap()[o * P:(o + 1) * P, :]
                    )
            for name in ("Wiz", "Uhz"):
                for c in range(2):
                    o0 = c * (NS // 2)
                    nc.sync.dma_start(
                        out=wsb[name][:, o0:o0 + NS // 2, :],
                        in_=wts[name].ap()[o0 * P:(o0 + NS // 2) * P, :]
                        .rearrange("(o p) n -> p o n", p=P),
                    )
            # ---- ACT queue: R weights per-o (early start), then biases
            # (needed at first R ACT), then Win/Uhn.
            for o in range(NS):
                for name in ("Wir", "Uhr"):
                    nc.scalar.dma_start(
                        out=wsb[name][:, o, :],
                        in_=wts[name].ap()[o * P:(o + 1) * P, :],
                    )
            for name in ("brt", "bzt", "bnt"):
                nc.scalar.dma_start(out=bias[name], in_=bts[name].ap())
            for name in ("Win", "Uhn"):
                for c in range(2):
                    o0 = c * (NS // 2)
                    nc.scalar.dma_start(
                        out=wsb[name][:, o0:o0 + NS // 2, :],
                        in_=wts[name].ap()[o0 * P:(o0 + NS // 2) * P, :]
                        .rearrange("(o p) n -> p o n", p=P),
                    )

            b0 = slice(0, BH)
            b1 = slice(BH, B_CORE)

            # ---- phase R: r = sig(x@Wir + h@Uhr + br); rh = r * hT
            # bh0 pass is o-outer across all 8 hs groups (8 PSUM banks) so
            # each arriving (x,Wir)-chunk o immediately unlocks 8 matmuls --
            # the PE is never blocked behind a group's o=7 chunk while the
            # DMA feed trickles in.
            ps_r = [psp.tile([P, BH], F32, tag="mm", name=f"psr{hs}")
                    for hs in range(NS)]
            for o in range(NS):
                for hs in range(NS):
                    nc.tensor.matmul(
                        ps_r[hs], wsb["Wir"][:, o, hs * P:(hs + 1) * P],
                        xT[:, o, b0], start=(o == 0), stop=False,
                    )
            for o in range(NS):
                for hs in range(NS):
                    nc.tensor.matmul(
                        ps_r[hs], wsb["Uhr"][:, o, hs * P:(hs + 1) * P],
                        hT[:, o, b0], start=False, stop=(o == NS - 1),
                    )
            for hs in range(NS):
                nc.scalar.activation(
                    rh[:, hs, b0], ps_r[hs], SIG, bias=bias["brt"][:, hs:hs + 1]
                )
                nc.vector.tensor_mul(
                    rh[:, hs, b0], rh[:, hs, b0], hT[:, hs, b0]
                )
            # bh1 pass: everything is resident by now; normal hs-outer groups
            for hs in range(NS):
                cs = slice(hs * P, (hs + 1) * P)
                ps = psp.tile([P, BH], F32, tag="mm")
                for o in range(NS):
                    nc.tensor.matmul(ps, wsb["Wir"][:, o, cs], xT[:, o, b1],
                                     start=(o == 0), stop=False)
                for o in range(NS):
                    nc.tensor.matmul(ps, wsb["Uhr"][:, o, cs], hT[:, o, b1],
                                     start=False, stop=(o == NS - 1))
                nc.scalar.activation(
                    rh[:, hs, b1], ps, SIG, bias=bias["brt"][:, hs:hs + 1]
                )
                nc.vector.tensor_mul(
                    rh[:, hs, b1], rh[:, hs, b1], hT[:, hs, b1]
                )

            # ---- phase ZN + combine
            for hs in range(NS):
                cs = slice(hs * P, (hs + 1) * P)
                ps_z0 = psp.tile([P, BH], F32, tag="mm")
                ps_z1 = psp.tile([P, BH], F32, tag="mm")
                ps_n0 = psp.tile([P, BH], F32, tag="mm")
                ps_n1 = psp.tile([P, BH], F32, tag="mm")
                for o in range(NS):
                    w = wsb["Wiz"][:, o, cs]
                    nc.tensor.matmul(ps_z0, w, xT[:, o, b0],
                                     start=(o == 0), stop=False)
                    nc.tensor.matmul(ps_z1, w, xT[:, o, b1],
                                     start=(o == 0), stop=False)
                for o in range(NS):
                    w = wsb["Uhz"][:, o, cs]
                    nc.tensor.matmul(ps_z0, w, hT[:, o, b0],
                                     start=False, stop=(o == NS - 1))
                    nc.tensor.matmul(ps_z1, w, hT[:, o, b1],
                                     start=False, stop=(o == NS - 1))
                z0 = gt.tile([P, BH], F16, tag="g")
                nc.scalar.activation(z0, ps_z0, SIG, bias=bias["bzt"][:, hs:hs + 1])
                z1 = gt.tile([P, BH], F16, tag="g")
                nc.scalar.activation(z1, ps_z1, SIG, bias=bias["bzt"][:, hs:hs + 1])
                for o in range(NS):
                    w = wsb["Win"][:, o, cs]
                    nc.tensor.matmul(ps_n0, w, xT[:, o, b0],
                                     start=(o == 0), stop=False)
                    nc.tensor.matmul(ps_n1, w, xT[:, o, b1],
                                     start=(o == 0), stop=False)
                for o in range(NS):
                    w = wsb["Uhn"][:, o, cs]
                    nc.tensor.matmul(ps_n0, w, rh[:, o, b0],
                                     start=False, stop=(o == NS - 1))
                    nc.tensor.matmul(ps_n1, w, rh[:, o, b1],
                                     start=False, stop=(o == NS - 1))
                # d = (n - h)*z + h, feature-major, fp16; per-half so the
                # b0 store overlaps the b1 sigmoid/DVE chain. The very last
                # drain (hs=7, b1) is further split in quarters to shorten
                # the post-matmul tail latency.
                for ps_n, bs, zt in ((ps_n0, b0, z0), (ps_n1, b1, z1)):
                    nq = 2 if (hs == NS - 1 and bs is b1) else 1
                    qw = BH // nq
                    for q in range(nq):
                        qs = slice(bs.start + q * qw, bs.start + (q + 1) * qw)
                        pq = slice(q * qw, (q + 1) * qw)
                        nt = gt.tile([P, qw], F16, tag=f"g{nq}")
                        nc.scalar.activation(
                            nt, ps_n[:, pq], SIG, bias=bias["bnt"][:, hs:hs + 1]
                        )
                        d_t = dp.tile([P, qw], F16, tag=f"d{nq}")
                        nc.vector.tensor_sub(d_t, nt, hT[:, hs, qs])
                        nc.vector.tensor_mul(d_t, d_t, zt[:, pq])
                        nc.vector.tensor_add(d_t, d_t, hT[:, hs, qs])
                        nc.sync.dma_start(out=out.ap()[cs, qs], in_=d_t)

    nc.compile()
    return nc


def _get_nc():
    if "nc" not in _NC_CACHE:
        _NC_CACHE["nc"] = _build_bass()
    return _NC_CACHE["nc"]


def make_in_maps(inputs):
    f16w = {
        k: np.ascontiguousarray(np.asarray(inputs[k], dtype=np.float16))
        for k in ("Wiz", "Uhz", "Wir", "Uhr", "Win", "Uhn")
    }
    shared = dict(f16w)
    for name, key in (("bzt", "bz"), ("brt", "br"), ("bnt", "bn")):
        shared[name] = np.ascontiguousarray(
            np.asarray(inputs[key], dtype=np.float32).reshape(NS, P).T
        )
    x16 = np.asarray(inputs["x"], dtype=np.float16)
    h16 = np.asarray(inputs["h"], dtype=np.float16)
    in_maps = []
    for c in range(N_CORES):
        sl = slice(c * B_CORE, (c + 1) * B_CORE)
        m = {
            "xt": np.ascontiguousarray(x16[sl].T),
            "ht": np.ascontiguousarray(h16[sl].T),
        }
        m.update(shared)
        in_maps.append(m)
    return in_maps


def kernel(**inputs):
    from concourse.bass_utils import run_bass_kernel_spmd

    nc = _get_nc()
    in_maps = make_in_maps(inputs)
    res = run_bass_kernel_spmd(nc, in_maps, list(range(N_CORES)))
    out = np.concatenate(
        [res.results[c]["out"].T for c in range(N_CORES)], axis=0
    )
    return out.astype(np.float32)


# revision 14
# speedup vs baseline: 1.0269x; 1.0269x over previous
"""Trainium2 Bass kernel for the GRUCell-variant problem.

  z = sigmoid(x@Wiz + h@Uhz + bz)
  r = sigmoid(x@Wir + h@Uhr + br)
  n = sigmoid(x@Win + (r*h)@Uhn + bn)
  out = (1-z)*h + z*n

Full shapes: x,h [8192,1024]; W*,U* [1024,1024]; b* [1024].
Sharding: data-parallel over batch across 8 NeuronCores (1024 rows each);
weights replicated; no collectives.

v3 design (fp16 compute, fp32 PSUM accumulate, zero device transposes):
  - Host pre-transposes x,h to feature-major [D, B_CORE] fp16 and biases
    to [128,8] fp32; weights are cast to fp16 in natural layout. The
    device output is feature-major [D, B_CORE] fp16; the host transposes
    back and upcasts. All layout shuffling is host-side numpy - the
    device does exclusively matmul + sigmoid + elementwise.
  - Everything SBUF-resident: xT,hT,rh + 6 weight matrices (fp16 halves
    the footprint so it all fits).
  - Matmuls are 768 x [128d x 128h stationary] @ [128d x 512b moving]
    fp16 (1 cyc/row): ~164us PE floor. Pairs of matmuls (batch half 0/1)
    share a stationary tile back-to-back.
  - R phase per-o weight chunks let the PE start ~1.5us in; ZN phase
    z-matmuls precede n-matmuls so rh (needs R complete) is never waited
    on.
"""

import sys

if "/opt/trn_rl_repo" not in sys.path:
    sys.path.insert(0, "/opt/trn_rl_repo")

import numpy as np

P = 128
B_FULL = 8192
D = 1024  # d_in == d_h == 1024
N_CORES = 8
B_CORE = B_FULL // N_CORES  # 1024
NS = D // P  # 8 strips of 128 along any 1024 dim
BH = 512  # moving free-dim per matmul (one PSUM bank in fp32)
NBH = B_CORE // BH  # 2 batch halves

_NC_CACHE = {}


def _build_bass():
    import concourse.mybir as mybir
    import concourse.tile as tile
    from concourse import bacc

    F16 = mybir.dt.float16
    F32 = mybir.dt.float32
    SIG = mybir.ActivationFunctionType.Sigmoid

    nc = bacc.Bacc(None, target_bir_lowering=False)

    # x,h arrive pre-transposed to feature-major [D, B] fp16
    xt = nc.dram_tensor("xt", [D, B_CORE], F16, kind="ExternalInput")
    ht = nc.dram_tensor("ht", [D, B_CORE], F16, kind="ExternalInput")
    wts = {
        name: nc.dram_tensor(name, [D, D], F16, kind="ExternalInput")
        for name in ("Wiz", "Uhz", "Wir", "Uhr", "Win", "Uhn")
    }
    bts = {
        name: nc.dram_tensor(name, [P, NS], F32, kind="ExternalInput")
        for name in ("bzt", "brt", "bnt")
    }
    # feature-major output; host transposes back
    out = nc.dram_tensor("out", [D, B_CORE], F16, kind="ExternalOutput")

    with tile.TileContext(nc) as tc:
        with (
            tc.tile_pool(name="big", bufs=1) as big,
            tc.tile_pool(name="dp", bufs=4) as dp,
            tc.tile_pool(name="gt", bufs=8) as gt,
            tc.tile_pool(name="ps", bufs=8, space="PSUM") as psp,
        ):
            # Persistent feature-major activations: [p, o, b] = val[o*128+p, b]
            xT = big.tile([P, NS, B_CORE], F16, tag="xT")
            hT = big.tile([P, NS, B_CORE], F16, tag="hT")
            rh = big.tile([P, NS, B_CORE], F16, tag="rh")
            # Weights natural layout: [p, o, n] = W[o*128+p, n]
            wsb = {}
            for name in ("Wir", "Uhr", "Wiz", "Uhz", "Win", "Uhn"):
                wsb[name] = big.tile(
                    [P, NS, D], F16, tag=f"w_{name}", name=f"w_{name}"
                )
            bias = {}
            for name in ("bzt", "brt", "bnt"):
                bt = big.tile([P, NS], F32, tag=name)
                bias[name] = bt

            # ---- inputs on SP queue: (x,h) chunk pairs per o (R-bh0
            # consumption order), then Wiz/Uhz for the ZN phase.
            for o in range(NS):
                for src, dst in ((xt, xT), (ht, hT)):
                    nc.sync.dma_start(
                        out=dst[:, o, :], in_=src.ap()[o * P:(o + 1) * P, :]
                    )
            for name in ("Wiz", "Uhz"):
                for c in range(2):
                    o0 = c * (NS // 2)
                    nc.sync.dma_start(
                        out=wsb[name][:, o0:o0 + NS // 2, :],
                        in_=wts[name].ap()[o0 * P:(o0 + NS // 2) * P, :]
                        .rearrange("(o p) n -> p o n", p=P),
                    )
            # ---- ACT queue: R weights per-o (early start), then biases
            # (needed at first R ACT), then Win/Uhn.
            for o in range(NS):
                for name in ("Wir", "Uhr"):
                    nc.scalar.dma_start(
                        out=wsb[name][:, o, :],
                        in_=wts[name].ap()[o * P:(o + 1) * P, :],
                    )
            for name in ("brt", "bzt", "bnt"):
                nc.scalar.dma_start(out=bias[name], in_=bts[name].ap())
            for name in ("Win", "Uhn"):
                for c in range(2):
                    o0 = c * (NS // 2)
                    nc.scalar.dma_start(
                        out=wsb[name][:, o0:o0 + NS // 2, :],
                        in_=wts[name].ap()[o0 * P:(o0 + NS // 2) * P, :]
                        .rearrange("(o p) n -> p o n", p=P),
                    )

            b0 = slice(0, BH)
            b1 = slice(BH, B_CORE)

            # ---- phase R: r = sig(x@Wir + h@Uhr + br); rh = r * hT
            # bh0 pass is o-outer across all 8 hs groups (8 PSUM banks) so
            # each arriving (x,Wir)-chunk o immediately unlocks 8 matmuls --
            # the PE is never blocked behind a group's o=7 chunk while the
            # DMA feed trickles in.
            ps_r = [psp.tile([P, BH], F32, tag="mm", name=f"psr{hs}")
                    for hs in range(NS)]
            for o in range(NS):
                for hs in range(NS):
                    nc.tensor.matmul(
                        ps_r[hs], wsb["Wir"][:, o, hs * P:(hs + 1) * P],
                        xT[:, o, b0], start=(o == 0), stop=False,
                    )
                for hs in range(NS):
                    nc.tensor.matmul(
                        ps_r[hs], wsb["Uhr"][:, o, hs * P:(hs + 1) * P],
                        hT[:, o, b0], start=False, stop=(o == NS - 1),
                    )
            for hs in range(NS):
                nc.scalar.activation(
                    rh[:, hs, b0], ps_r[hs], SIG, bias=bias["brt"][:, hs:hs + 1]
                )
                nc.vector.tensor_mul(
                    rh[:, hs, b0], rh[:, hs, b0], hT[:, hs, b0]
                )
            # bh1 pass: everything is resident by now; normal hs-outer groups
            for hs in range(NS):
                cs = slice(hs * P, (hs + 1) * P)
                ps = psp.tile([P, BH], F32, tag="mm")
                for o in range(NS):
                    nc.tensor.matmul(ps, wsb["Wir"][:, o, cs], xT[:, o, b1],
                                     start=(o == 0), stop=False)
                for o in range(NS):
                    nc.tensor.matmul(ps, wsb["Uhr"][:, o, cs], hT[:, o, b1],
                                     start=False, stop=(o == NS - 1))
                nc.scalar.activation(
                    rh[:, hs, b1], ps, SIG, bias=bias["brt"][:, hs:hs + 1]
                )
                nc.vector.tensor_mul(
                    rh[:, hs, b1], rh[:, hs, b1], hT[:, hs, b1]
                )

            # ---- phase ZN + combine
            for hs in range(NS):
                cs = slice(hs * P, (hs + 1) * P)
                ps_z0 = psp.tile([P, BH], F32, tag="mm")
                ps_z1 = psp.tile([P, BH], F32, tag="mm")
                ps_n0 = psp.tile([P, BH], F32, tag="mm")
                ps_n1 = psp.tile([P, BH], F32, tag="mm")
                for o in range(NS):
                    w = wsb["Wiz"][:, o, cs]
                    nc.tensor.matmul(ps_z0, w, xT[:, o, b0],
                                     start=(o == 0), stop=False)
                    nc.tensor.matmul(ps_z1, w, xT[:, o, b1],
                                     start=(o == 0), stop=False)
                for o in range(NS):
                    w = wsb["Uhz"][:, o, cs]
                    nc.tensor.matmul(ps_z0, w, hT[:, o, b0],
                                     start=False, stop=(o == NS - 1))
                    nc.tensor.matmul(ps_z1, w, hT[:, o, b1],
                                     start=False, stop=(o == NS - 1))
                z0 = gt.tile([P, BH], F16, tag="g")
                nc.scalar.activation(z0, ps_z0, SIG, bias=bias["bzt"][:, hs:hs + 1])
                z1 = gt.tile([P, BH], F16, tag="g")
                nc.scalar.activation(z1, ps_z1, SIG, bias=bias["bzt"][:, hs:hs + 1])
                for o in range(NS):
                    w = wsb["Win"][:, o, cs]
                    nc.tensor.matmul(ps_n0, w, xT[:, o, b0],
                                     start=(o == 0), stop=False)
                    nc.tensor.matmul(ps_n1, w, xT[:, o, b1],
                                     start=(o == 0), stop=False)
                for o in range(NS):
                    w = wsb["Uhn"][:, o, cs]
                    nc.tensor.matmul(ps_n0, w, rh[:, o, b0],
                                     start=False, stop=(o == NS - 1))
                    nc.tensor.matmul(ps_n1, w, rh[:, o, b1],
                                     start=False, stop=(o == NS - 1))
                # d = (n - h)*z + h, feature-major, fp16; per-half so the
                # b0 store overlaps the b1 sigmoid/DVE chain. The very last
                # drain (hs=7, b1) is further split in quarters to shorten
                # the post-matmul tail latency.
                for ps_n, bs, zt in ((ps_n0, b0, z0), (ps_n1, b1, z1)):
                    nq = 2 if (hs == NS - 1 and bs is b1) else 1
                    qw = BH // nq
                    for q in range(nq):
                        qs = slice(bs.start + q * qw, bs.start + (q + 1) * qw)
                        pq = slice(q * qw, (q + 1) * qw)
                        nt = gt.tile([P, qw], F16, tag=f"g{nq}")
                        nc.scalar.activation(
                            nt, ps_n[:, pq], SIG, bias=bias["bnt"][:, hs:hs + 1]
                        )
                        d_t = dp.tile([P, qw], F16, tag=f"d{nq}")
                        nc.vector.tensor_sub(d_t, nt, hT[:, hs, qs])
                        nc.vector.tensor_mul(d_t, d_t, zt[:, pq])
                        nc.vector.tensor_add(d_t, d_t, hT[:, hs, qs])
                        # final quarter-store goes on the idle ACT queue so
                        # the two tail stores dispatch in parallel
                        eng = nc.scalar if (nq == 2 and q == 1) else nc.sync
                        eng.dma_start(out=out.ap()[cs, qs], in_=d_t)

    nc.compile()
    return nc


def _get_nc():
    if "nc" not in _NC_CACHE:
        _NC_CACHE["nc"] = _build_bass()
    return _NC_CACHE["nc"]


def make_in_maps(inputs):
    f16w = {
        k: np.ascontiguousarray(np.asarray(inputs[k], dtype=np.float16))
        for k in ("Wiz", "Uhz", "Wir", "Uhr", "Win", "Uhn")
    }
    shared = dict(f16w)
    for name, key in (("bzt", "bz"), ("brt", "br"), ("bnt", "bn")):
        shared[name] = np.ascontiguousarray(
            np.asarray(inputs[key], dtype=np.float32).reshape(NS, P).T
        )
    x16 = np.asarray(inputs["x"], dtype=np.float16)
    h16 = np.asarray(inputs["h"], dtype=np.float16)
    in_maps = []
    for c in range(N_CORES):
        sl = slice(c * B_CORE, (c + 1) * B_CORE)
        m = {
            "xt": np.ascontiguousarray(x16[sl].T),
            "ht": np.ascontiguousarray(h16[sl].T),
        }
        m.update(shared)
        in_maps.append(m)
    return in_maps


def kernel(**inputs):
    from concourse.bass_utils import run_bass_kernel_spmd

    nc = _get_nc()
    in_maps = make_in_maps(inputs)
    res = run_bass_kernel_spmd(nc, in_maps, list(range(N_CORES)))
    out = np.concatenate(
        [res.results[c]["out"].T for c in range(N_CORES)], axis=0
    )
    return out.astype(np.float32)


# revision 15
# speedup vs baseline: 1.0501x; 1.0226x over previous
"""Trainium2 Bass kernel for the GRUCell-variant problem.

  z = sigmoid(x@Wiz + h@Uhz + bz)
  r = sigmoid(x@Wir + h@Uhr + br)
  n = sigmoid(x@Win + (r*h)@Uhn + bn)
  out = (1-z)*h + z*n

Full shapes: x,h [8192,1024]; W*,U* [1024,1024]; b* [1024].
Sharding: data-parallel over batch across 8 NeuronCores (1024 rows each);
weights replicated; no collectives.

v3 design (fp16 compute, fp32 PSUM accumulate, zero device transposes):
  - Host pre-transposes x,h to feature-major [D, B_CORE] fp16 and biases
    to [128,8] fp32; weights are cast to fp16 in natural layout. The
    device output is feature-major [D, B_CORE] fp16; the host transposes
    back and upcasts. All layout shuffling is host-side numpy - the
    device does exclusively matmul + sigmoid + elementwise.
  - Everything SBUF-resident: xT,hT,rh + 6 weight matrices (fp16 halves
    the footprint so it all fits).
  - Matmuls are 768 x [128d x 128h stationary] @ [128d x 512b moving]
    fp16 (1 cyc/row): ~164us PE floor. Pairs of matmuls (batch half 0/1)
    share a stationary tile back-to-back.
  - R phase per-o weight chunks let the PE start ~1.5us in; ZN phase
    z-matmuls precede n-matmuls so rh (needs R complete) is never waited
    on.
"""

import sys

if "/opt/trn_rl_repo" not in sys.path:
    sys.path.insert(0, "/opt/trn_rl_repo")

import numpy as np

P = 128
B_FULL = 8192
D = 1024  # d_in == d_h == 1024
N_CORES = 8
B_CORE = B_FULL // N_CORES  # 1024
NS = D // P  # 8 strips of 128 along any 1024 dim
BH = 512  # moving free-dim per matmul (one PSUM bank in fp32)
NBH = B_CORE // BH  # 2 batch halves

_NC_CACHE = {}


def _build_bass():
    import concourse.mybir as mybir
    import concourse.tile as tile
    from concourse import bacc

    F16 = mybir.dt.float16
    F32 = mybir.dt.float32
    SIG = mybir.ActivationFunctionType.Sigmoid

    nc = bacc.Bacc(None, target_bir_lowering=False)

    # x,h arrive pre-transposed to feature-major [D, B] fp16
    xt = nc.dram_tensor("xt", [D, B_CORE], F16, kind="ExternalInput")
    ht = nc.dram_tensor("ht", [D, B_CORE], F16, kind="ExternalInput")
    wts = {
        name: nc.dram_tensor(name, [D, D], F16, kind="ExternalInput")
        for name in ("Wiz", "Uhz", "Wir", "Uhr", "Win", "Uhn")
    }
    bts = {
        name: nc.dram_tensor(name, [P, NS], F32, kind="ExternalInput")
        for name in ("bzt", "brt", "bnt")
    }
    # feature-major output; host transposes back
    out = nc.dram_tensor("out", [D, B_CORE], F16, kind="ExternalOutput")

    with tile.TileContext(nc) as tc:
        with (
            tc.tile_pool(name="big", bufs=1) as big,
            tc.tile_pool(name="dp", bufs=4) as dp,
            tc.tile_pool(name="gt", bufs=8) as gt,
            tc.tile_pool(name="ps", bufs=8, space="PSUM") as psp,
        ):
            # Persistent feature-major activations: [p, o, b] = val[o*128+p, b]
            xT = big.tile([P, NS, B_CORE], F16, tag="xT")
            hT = big.tile([P, NS, B_CORE], F16, tag="hT")
            rh = big.tile([P, NS, B_CORE], F16, tag="rh")
            # Weights natural layout: [p, o, n] = W[o*128+p, n]
            wsb = {}
            for name in ("Wir", "Uhr", "Wiz", "Uhz", "Win", "Uhn"):
                wsb[name] = big.tile(
                    [P, NS, D], F16, tag=f"w_{name}", name=f"w_{name}"
                )
            bias = {}
            for name in ("bzt", "brt", "bnt"):
                bt = big.tile([P, NS], F32, tag=name)
                bias[name] = bt

            # ---- inputs on SP queue: (x,h) chunk pairs per o (R-bh0
            # consumption order), then Wiz/Uhz for the ZN phase.
            for o in range(NS):
                for src, dst in ((xt, xT), (ht, hT)):
                    nc.sync.dma_start(
                        out=dst[:, o, :], in_=src.ap()[o * P:(o + 1) * P, :]
                    )
            # All ZN weights also go on the SP queue: it runs no compute, so
            # long transfers never block a drain. The ACT queue carries only
            # the R-critical Wir/Uhr chunks + biases, then is free for
            # sigmoids.
            for name in ("Wiz", "Uhz", "Win", "Uhn"):
                for c in range(2):
                    o0 = c * (NS // 2)
                    nc.sync.dma_start(
                        out=wsb[name][:, o0:o0 + NS // 2, :],
                        in_=wts[name].ap()[o0 * P:(o0 + NS // 2) * P, :]
                        .rearrange("(o p) n -> p o n", p=P),
                    )
            for o in range(NS):
                for name in ("Wir", "Uhr"):
                    nc.scalar.dma_start(
                        out=wsb[name][:, o, :],
                        in_=wts[name].ap()[o * P:(o + 1) * P, :],
                    )
            for name in ("brt", "bzt", "bnt"):
                nc.scalar.dma_start(out=bias[name], in_=bts[name].ap())

            b0 = slice(0, BH)
            b1 = slice(BH, B_CORE)

            # ---- phase R: r = sig(x@Wir + h@Uhr + br); rh = r * hT
            # bh0 pass is o-outer across all 8 hs groups (8 PSUM banks) so
            # each arriving (x,Wir)-chunk o immediately unlocks 8 matmuls --
            # the PE is never blocked behind a group's o=7 chunk while the
            # DMA feed trickles in.
            ps_r = [psp.tile([P, BH], F32, tag="mm", name=f"psr{hs}")
                    for hs in range(NS)]
            for o in range(NS):
                for hs in range(NS):
                    nc.tensor.matmul(
                        ps_r[hs], wsb["Wir"][:, o, hs * P:(hs + 1) * P],
                        xT[:, o, b0], start=(o == 0), stop=False,
                    )
                for hs in range(NS):
                    nc.tensor.matmul(
                        ps_r[hs], wsb["Uhr"][:, o, hs * P:(hs + 1) * P],
                        hT[:, o, b0], start=False, stop=(o == NS - 1),
                    )
            for hs in range(NS):
                nc.scalar.activation(
                    rh[:, hs, b0], ps_r[hs], SIG, bias=bias["brt"][:, hs:hs + 1]
                )
                nc.vector.tensor_mul(
                    rh[:, hs, b0], rh[:, hs, b0], hT[:, hs, b0]
                )
            # bh1 pass: everything is resident by now; normal hs-outer groups
            for hs in range(NS):
                cs = slice(hs * P, (hs + 1) * P)
                ps = psp.tile([P, BH], F32, tag="mm")
                for o in range(NS):
                    nc.tensor.matmul(ps, wsb["Wir"][:, o, cs], xT[:, o, b1],
                                     start=(o == 0), stop=False)
                for o in range(NS):
                    nc.tensor.matmul(ps, wsb["Uhr"][:, o, cs], hT[:, o, b1],
                                     start=False, stop=(o == NS - 1))
                nc.scalar.activation(
                    rh[:, hs, b1], ps, SIG, bias=bias["brt"][:, hs:hs + 1]
                )
                nc.vector.tensor_mul(
                    rh[:, hs, b1], rh[:, hs, b1], hT[:, hs, b1]
                )

            # ---- phase ZN + combine
            for hs in range(NS):
                cs = slice(hs * P, (hs + 1) * P)
                ps_z0 = psp.tile([P, BH], F32, tag="mm")
                ps_z1 = psp.tile([P, BH], F32, tag="mm")
                ps_n0 = psp.tile([P, BH], F32, tag="mm")
                ps_n1 = psp.tile([P, BH], F32, tag="mm")
                for o in range(NS):
                    w = wsb["Wiz"][:, o, cs]
                    nc.tensor.matmul(ps_z0, w, xT[:, o, b0],
                                     start=(o == 0), stop=False)
                    nc.tensor.matmul(ps_z1, w, xT[:, o, b1],
                                     start=(o == 0), stop=False)
                for o in range(NS):
                    w = wsb["Uhz"][:, o, cs]
                    nc.tensor.matmul(ps_z0, w, hT[:, o, b0],
                                     start=False, stop=(o == NS - 1))
                    nc.tensor.matmul(ps_z1, w, hT[:, o, b1],
                                     start=False, stop=(o == NS - 1))
                z0 = gt.tile([P, BH], F16, tag="g")
                nc.scalar.activation(z0, ps_z0, SIG, bias=bias["bzt"][:, hs:hs + 1])
                z1 = gt.tile([P, BH], F16, tag="g")
                nc.scalar.activation(z1, ps_z1, SIG, bias=bias["bzt"][:, hs:hs + 1])
                for o in range(NS):
                    w = wsb["Win"][:, o, cs]
                    nc.tensor.matmul(ps_n0, w, xT[:, o, b0],
                                     start=(o == 0), stop=False)
                    nc.tensor.matmul(ps_n1, w, xT[:, o, b1],
                                     start=(o == 0), stop=False)
                for o in range(NS):
                    w = wsb["Uhn"][:, o, cs]
                    nc.tensor.matmul(ps_n0, w, rh[:, o, b0],
                                     start=False, stop=(o == NS - 1))
                    nc.tensor.matmul(ps_n1, w, rh[:, o, b1],
                                     start=False, stop=(o == NS - 1))
                # d = (n - h)*z + h, feature-major, fp16; per-half so the
                # b0 store overlaps the b1 sigmoid/DVE chain. The very last
                # drain (hs=7, b1) is further split in quarters to shorten
                # the post-matmul tail latency.
                for ps_n, bs, zt in ((ps_n0, b0, z0), (ps_n1, b1, z1)):
                    nq = 2 if (hs == NS - 1 and bs is b1) else 1
                    qw = BH // nq
                    for q in range(nq):
                        qs = slice(bs.start + q * qw, bs.start + (q + 1) * qw)
                        pq = slice(q * qw, (q + 1) * qw)
                        nt = gt.tile([P, qw], F16, tag=f"g{nq}")
                        nc.scalar.activation(
                            nt, ps_n[:, pq], SIG, bias=bias["bnt"][:, hs:hs + 1]
                        )
                        d_t = dp.tile([P, qw], F16, tag=f"d{nq}")
                        nc.vector.tensor_sub(d_t, nt, hT[:, hs, qs])
                        nc.vector.tensor_mul(d_t, d_t, zt[:, pq])
                        nc.vector.tensor_add(d_t, d_t, hT[:, hs, qs])
                        # final quarter-store goes on the idle ACT queue so
                        # the two tail stores dispatch in parallel
                        eng = nc.scalar if (nq == 2 and q == 1) else nc.sync
                        eng.dma_start(out=out.ap()[cs, qs], in_=d_t)

    nc.compile()
    return nc


def _get_nc():
    if "nc" not in _NC_CACHE:
        _NC_CACHE["nc"] = _build_bass()
    return _NC_CACHE["nc"]


def make_in_maps(inputs):
    f16w = {
        k: np.ascontiguousarray(np.asarray(inputs[k], dtype=np.float16))
        for k in ("Wiz", "Uhz", "Wir", "Uhr", "Win", "Uhn")
    }
    shared = dict(f16w)
    for name, key in (("bzt", "bz"), ("brt", "br"), ("bnt", "bn")):
        shared[name] = np.ascontiguousarray(
            np.asarray(inputs[key], dtype=np.float32).reshape(NS, P).T
        )
    x16 = np.asarray(inputs["x"], dtype=np.float16)
    h16 = np.asarray(inputs["h"], dtype=np.float16)
    in_maps = []
    for c in range(N_CORES):
        sl = slice(c * B_CORE, (c + 1) * B_CORE)
        m = {
            "xt": np.ascontiguousarray(x16[sl].T),
            "ht": np.ascontiguousarray(h16[sl].T),
        }
        m.update(shared)
        in_maps.append(m)
    return in_maps


def kernel(**inputs):
    from concourse.bass_utils import run_bass_kernel_spmd

    nc = _get_nc()
    in_maps = make_in_maps(inputs)
    res = run_bass_kernel_spmd(nc, in_maps, list(range(N_CORES)))
    out = np.concatenate(
        [res.results[c]["out"].T for c in range(N_CORES)], axis=0
    )
    return out.astype(np.float32)


# revision 18
# speedup vs baseline: 1.0579x; 1.0074x over previous
"""Trainium2 Bass kernel for the GRUCell-variant problem.

  z = sigmoid(x@Wiz + h@Uhz + bz)
  r = sigmoid(x@Wir + h@Uhr + br)
  n = sigmoid(x@Win + (r*h)@Uhn + bn)
  out = (1-z)*h + z*n

Full shapes: x,h [8192,1024]; W*,U* [1024,1024]; b* [1024].
Sharding: data-parallel over batch across 8 NeuronCores (1024 rows each);
weights replicated; no collectives.

v3 design (fp16 compute, fp32 PSUM accumulate, zero device transposes):
  - Host pre-transposes x,h to feature-major [D, B_CORE] fp16 and biases
    to [128,8] fp32; weights are cast to fp16 in natural layout. The
    device output is feature-major [D, B_CORE] fp16; the host transposes
    back and upcasts. All layout shuffling is host-side numpy - the
    device does exclusively matmul + sigmoid + elementwise.
  - Everything SBUF-resident: xT,hT,rh + 6 weight matrices (fp16 halves
    the footprint so it all fits).
  - Matmuls are 768 x [128d x 128h stationary] @ [128d x 512b moving]
    fp16 (1 cyc/row): ~164us PE floor. Pairs of matmuls (batch half 0/1)
    share a stationary tile back-to-back.
  - R phase per-o weight chunks let the PE start ~1.5us in; ZN phase
    z-matmuls precede n-matmuls so rh (needs R complete) is never waited
    on.
"""

import sys

if "/opt/trn_rl_repo" not in sys.path:
    sys.path.insert(0, "/opt/trn_rl_repo")

import numpy as np

P = 128
B_FULL = 8192
D = 1024  # d_in == d_h == 1024
N_CORES = 8
B_CORE = B_FULL // N_CORES  # 1024
NS = D // P  # 8 strips of 128 along any 1024 dim
BH = 512  # moving free-dim per matmul (one PSUM bank in fp32)
NBH = B_CORE // BH  # 2 batch halves

_NC_CACHE = {}


def _build_bass():
    import concourse.mybir as mybir
    import concourse.tile as tile
    from concourse import bacc

    F16 = mybir.dt.float16
    F32 = mybir.dt.float32
    SIG = mybir.ActivationFunctionType.Sigmoid

    nc = bacc.Bacc(None, target_bir_lowering=False)

    # x,h arrive pre-transposed to feature-major [D, B] fp16
    xt = nc.dram_tensor("xt", [D, B_CORE], F16, kind="ExternalInput")
    ht = nc.dram_tensor("ht", [D, B_CORE], F16, kind="ExternalInput")
    wts = {
        name: nc.dram_tensor(name, [D, D], F16, kind="ExternalInput")
        for name in ("Wiz", "Uhz", "Wir", "Uhr", "Win", "Uhn")
    }
    bts = {
        name: nc.dram_tensor(name, [P, NS], F32, kind="ExternalInput")
        for name in ("bzt", "brt", "bnt")
    }
    # feature-major output; host transposes back
    out = nc.dram_tensor("out", [D, B_CORE], F16, kind="ExternalOutput")

    with tile.TileContext(nc) as tc:
        with (
            tc.tile_pool(name="big", bufs=1) as big,
            tc.tile_pool(name="dp", bufs=4) as dp,
            tc.tile_pool(name="gt", bufs=8) as gt,
            tc.tile_pool(name="ps", bufs=8, space="PSUM") as psp,
        ):
            # Persistent feature-major activations: [p, o, b] = val[o*128+p, b]
            xT = big.tile([P, NS, B_CORE], F16, tag="xT")
            hT = big.tile([P, NS, B_CORE], F16, tag="hT")
            rh = big.tile([P, NS, B_CORE], F16, tag="rh")
            # Weights natural layout: [p, o, n] = W[o*128+p, n]
            wsb = {}
            for name in ("Wir", "Uhr", "Wiz", "Uhz", "Win", "Uhn"):
                wsb[name] = big.tile(
                    [P, NS, D], F16, tag=f"w_{name}", name=f"w_{name}"
                )
            bias = {}
            for name in ("bzt", "brt", "bnt"):
                bt = big.tile([P, NS], F32, tag=name)
                bias[name] = bt

            # ---- inputs on SP queue: (x,h) chunk pairs per o (R-bh0
            # consumption order), then Wiz/Uhz for the ZN phase.
            for o in range(NS):
                for src, dst in ((xt, xT), (ht, hT)):
                    nc.sync.dma_start(
                        out=dst[:, o, :], in_=src.ap()[o * P:(o + 1) * P, :]
                    )
            # All ZN weights also go on the SP queue: it runs no compute, so
            # long transfers never block a drain. The ACT queue carries only
            # the R-critical Wir/Uhr chunks + biases, then is free for
            # sigmoids.
            for name in ("Wiz", "Uhz", "Win", "Uhn"):
                for c in range(2):
                    o0 = c * (NS // 2)
                    nc.sync.dma_start(
                        out=wsb[name][:, o0:o0 + NS // 2, :],
                        in_=wts[name].ap()[o0 * P:(o0 + NS // 2) * P, :]
                        .rearrange("(o p) n -> p o n", p=P),
                    )
            for o in range(NS):
                for name in ("Wir", "Uhr"):
                    nc.scalar.dma_start(
                        out=wsb[name][:, o, :],
                        in_=wts[name].ap()[o * P:(o + 1) * P, :],
                    )
            for name in ("brt", "bzt", "bnt"):
                nc.scalar.dma_start(out=bias[name], in_=bts[name].ap())

            b0 = slice(0, BH)
            b1 = slice(BH, B_CORE)

            # ---- PE warmup: a few matmuls on zeroed scratch while the
            # first DMA chunks are still in flight, so the tensor engine's
            # DVFS is already ramped when the real stream begins.
            scratch = big.tile([P, BH], F16, tag="scratch", name="scratch")
            nc.vector.memset(scratch, 0.0)
            ps_r = [psp.tile([P, BH], F32, tag="mm", name=f"psr{hs}")
                    for hs in range(NS)]
            for _ in range(4):
                nc.tensor.matmul(ps_r[0], scratch[:, 0:P], scratch,
                                 start=True, stop=True)

            # ---- phase R: r = sig(x@Wir + h@Uhr + br); rh = r * hT
            # bh0 pass is o-outer across all 8 hs groups (8 PSUM banks) so
            # each arriving (x,Wir)-chunk o immediately unlocks 8 matmuls --
            # the PE is never blocked behind a group's o=7 chunk while the
            # DMA feed trickles in.
            for o in range(NS):
                for hs in range(NS):
                    nc.tensor.matmul(
                        ps_r[hs], wsb["Wir"][:, o, hs * P:(hs + 1) * P],
                        xT[:, o, b0], start=(o == 0), stop=False,
                    )
                for hs in range(NS):
                    nc.tensor.matmul(
                        ps_r[hs], wsb["Uhr"][:, o, hs * P:(hs + 1) * P],
                        hT[:, o, b0], start=False, stop=(o == NS - 1),
                    )
            for hs in range(NS):
                nc.scalar.activation(
                    rh[:, hs, b0], ps_r[hs], SIG, bias=bias["brt"][:, hs:hs + 1]
                )
                nc.vector.tensor_mul(
                    rh[:, hs, b0], rh[:, hs, b0], hT[:, hs, b0]
                )
            # bh1 pass: everything is resident by now; normal hs-outer groups
            for hs in range(NS):
                cs = slice(hs * P, (hs + 1) * P)
                ps = psp.tile([P, BH], F32, tag="mm")
                for o in range(NS):
                    nc.tensor.matmul(ps, wsb["Wir"][:, o, cs], xT[:, o, b1],
                                     start=(o == 0), stop=False)
                for o in range(NS):
                    nc.tensor.matmul(ps, wsb["Uhr"][:, o, cs], hT[:, o, b1],
                                     start=False, stop=(o == NS - 1))
                nc.scalar.activation(
                    rh[:, hs, b1], ps, SIG, bias=bias["brt"][:, hs:hs + 1]
                )
                nc.vector.tensor_mul(
                    rh[:, hs, b1], rh[:, hs, b1], hT[:, hs, b1]
                )

            # ---- phase ZN + combine
            for hs in range(NS):
                cs = slice(hs * P, (hs + 1) * P)
                ps_z0 = psp.tile([P, BH], F32, tag="mm")
                ps_z1 = psp.tile([P, BH], F32, tag="mm")
                ps_n0 = psp.tile([P, BH], F32, tag="mm")
                ps_n1 = psp.tile([P, BH], F32, tag="mm")
                for o in range(NS):
                    w = wsb["Wiz"][:, o, cs]
                    nc.tensor.matmul(ps_z0, w, xT[:, o, b0],
                                     start=(o == 0), stop=False)
                    nc.tensor.matmul(ps_z1, w, xT[:, o, b1],
                                     start=(o == 0), stop=False)
                for o in range(NS):
                    w = wsb["Uhz"][:, o, cs]
                    nc.tensor.matmul(ps_z0, w, hT[:, o, b0],
                                     start=False, stop=(o == NS - 1))
                    nc.tensor.matmul(ps_z1, w, hT[:, o, b1],
                                     start=False, stop=(o == NS - 1))
                z0 = gt.tile([P, BH], F16, tag="g")
                nc.scalar.activation(z0, ps_z0, SIG, bias=bias["bzt"][:, hs:hs + 1])
                z1 = gt.tile([P, BH], F16, tag="g")
                nc.scalar.activation(z1, ps_z1, SIG, bias=bias["bzt"][:, hs:hs + 1])
                for o in range(NS):
                    w = wsb["Win"][:, o, cs]
                    nc.tensor.matmul(ps_n0, w, xT[:, o, b0],
                                     start=(o == 0), stop=False)
                    nc.tensor.matmul(ps_n1, w, xT[:, o, b1],
                                     start=(o == 0), stop=False)
                for o in range(NS):
                    w = wsb["Uhn"][:, o, cs]
                    nc.tensor.matmul(ps_n0, w, rh[:, o, b0],
                                     start=False, stop=(o == NS - 1))
                    nc.tensor.matmul(ps_n1, w, rh[:, o, b1],
                                     start=False, stop=(o == NS - 1))
                # d = (n - h)*z + h, feature-major, fp16; per-half so the
                # b0 store overlaps the b1 sigmoid/DVE chain. The very last
                # drain (hs=7, b1) is further split in quarters to shorten
                # the post-matmul tail latency.
                for ps_n, bs, zt in ((ps_n0, b0, z0), (ps_n1, b1, z1)):
                    nq = 4 if (hs == NS - 1 and bs is b1) else 1
                    qw = BH // nq
                    for q in range(nq):
                        qs = slice(bs.start + q * qw, bs.start + (q + 1) * qw)
                        pq = slice(q * qw, (q + 1) * qw)
                        nt = gt.tile([P, qw], F16, tag=f"g{nq}")
                        nc.scalar.activation(
                            nt, ps_n[:, pq], SIG, bias=bias["bnt"][:, hs:hs + 1]
                        )
                        d_t = dp.tile([P, qw], F16, tag=f"d{nq}")
                        nc.vector.tensor_sub(d_t, nt, hT[:, hs, qs])
                        nc.vector.tensor_mul(d_t, d_t, zt[:, pq])
                        nc.vector.tensor_add(d_t, d_t, hT[:, hs, qs])
                        # alternate tail stores across both DMA queues so
                        # they dispatch in parallel
                        eng = nc.scalar if (nq > 1 and q % 2 == 1) else nc.sync
                        eng.dma_start(out=out.ap()[cs, qs], in_=d_t)

    nc.compile()
    return nc


def _get_nc():
    if "nc" not in _NC_CACHE:
        _NC_CACHE["nc"] = _build_bass()
    return _NC_CACHE["nc"]


def make_in_maps(inputs):
    f16w = {
        k: np.ascontiguousarray(np.asarray(inputs[k], dtype=np.float16))
        for k in ("Wiz", "Uhz", "Wir", "Uhr", "Win", "Uhn")
    }
    shared = dict(f16w)
    for name, key in (("bzt", "bz"), ("brt", "br"), ("bnt", "bn")):
        shared[name] = np.ascontiguousarray(
            np.asarray(inputs[key], dtype=np.float32).reshape(NS, P).T
        )
    x16 = np.asarray(inputs["x"], dtype=np.float16)
    h16 = np.asarray(inputs["h"], dtype=np.float16)
    in_maps = []
    for c in range(N_CORES):
        sl = slice(c * B_CORE, (c + 1) * B_CORE)
        m = {
            "xt": np.ascontiguousarray(x16[sl].T),
            "ht": np.ascontiguousarray(h16[sl].T),
        }
        m.update(shared)
        in_maps.append(m)
    return in_maps


def kernel(**inputs):
    from concourse.bass_utils import run_bass_kernel_spmd

    nc = _get_nc()
    in_maps = make_in_maps(inputs)
    res = run_bass_kernel_spmd(nc, in_maps, list(range(N_CORES)))
    out = np.concatenate(
        [res.results[c]["out"].T for c in range(N_CORES)], axis=0
    )
    return out.astype(np.float32)


# revision 19
# speedup vs baseline: 1.0587x; 1.0007x over previous
"""Trainium2 Bass kernel for the GRUCell-variant problem.

  z = sigmoid(x@Wiz + h@Uhz + bz)
  r = sigmoid(x@Wir + h@Uhr + br)
  n = sigmoid(x@Win + (r*h)@Uhn + bn)
  out = (1-z)*h + z*n

Full shapes: x,h [8192,1024]; W*,U* [1024,1024]; b* [1024].
Sharding: data-parallel over batch across 8 NeuronCores (1024 rows each);
weights replicated; no collectives.

Design (fp16 compute, fp32 PSUM accumulate, zero device transposes):
  - Host pre-transposes x,h to feature-major [D, B_CORE] fp16 and biases
    to [128,8] fp32; weights are cast to fp16 in natural layout. The
    device output is feature-major [D, B_CORE] fp16; the host transposes
    back and upcasts. All layout shuffling is host-side numpy - the
    device does exclusively matmul + sigmoid + elementwise (median rel
    err ~3.5e-4 vs the fp32 reference).
  - Everything SBUF-resident: xT,hT,rh + 6 weight matrices (fp16 halves
    the footprint so it all fits in ~21MiB of SBUF).
  - 768 matmuls of [128d x 128h stationary] @ [128d x 512b moving] fp16
    (1 cyc/row, ~216ns each): ~166us PE floor, and nothing else runs on
    the PE.
  - DMA queues: SP carries inputs + ZN weights + stores (it runs no
    compute, so a long transfer can never block a PSUM drain); ACT
    carries only the R-critical Wir/Uhr chunks + biases, then is free
    for sigmoids.
  - R-phase batch-half-0 pass is o-outer across all 8 hs PSUM groups so
    each arriving 0.25MiB chunk immediately unlocks 8 matmuls; the PE
    stream measures gapless on HW from first matmul to last.
  - 4 warmup matmuls on zeroed scratch ramp the PE DVFS while the first
    chunks are still in flight; the final drain is split in quarters
    with stores alternated across both DMA queues to shorten the tail.
Measured: 186.7-188.1us HW exec vs 249.0us for the fp32r baseline.
"""

import sys

if "/opt/trn_rl_repo" not in sys.path:
    sys.path.insert(0, "/opt/trn_rl_repo")

import numpy as np

P = 128
B_FULL = 8192
D = 1024  # d_in == d_h == 1024
N_CORES = 8
B_CORE = B_FULL // N_CORES  # 1024
NS = D // P  # 8 strips of 128 along any 1024 dim
BH = 512  # moving free-dim per matmul (one PSUM bank in fp32)
NBH = B_CORE // BH  # 2 batch halves

_NC_CACHE = {}


def _build_bass():
    import concourse.mybir as mybir
    import concourse.tile as tile
    from concourse import bacc

    F16 = mybir.dt.float16
    F32 = mybir.dt.float32
    SIG = mybir.ActivationFunctionType.Sigmoid

    nc = bacc.Bacc(None, target_bir_lowering=False)

    # x,h arrive pre-transposed to feature-major [D, B] fp16
    xt = nc.dram_tensor("xt", [D, B_CORE], F16, kind="ExternalInput")
    ht = nc.dram_tensor("ht", [D, B_CORE], F16, kind="ExternalInput")
    wts = {
        name: nc.dram_tensor(name, [D, D], F16, kind="ExternalInput")
        for name in ("Wiz", "Uhz", "Wir", "Uhr", "Win", "Uhn")
    }
    bts = {
        name: nc.dram_tensor(name, [P, NS], F32, kind="ExternalInput")
        for name in ("bzt", "brt", "bnt")
    }
    # feature-major output; host transposes back
    out = nc.dram_tensor("out", [D, B_CORE], F16, kind="ExternalOutput")

    with tile.TileContext(nc) as tc:
        with (
            tc.tile_pool(name="big", bufs=1) as big,
            tc.tile_pool(name="dp", bufs=4) as dp,
            tc.tile_pool(name="gt", bufs=8) as gt,
            tc.tile_pool(name="ps", bufs=8, space="PSUM") as psp,
        ):
            # Persistent feature-major activations: [p, o, b] = val[o*128+p, b]
            xT = big.tile([P, NS, B_CORE], F16, tag="xT")
            hT = big.tile([P, NS, B_CORE], F16, tag="hT")
            rh = big.tile([P, NS, B_CORE], F16, tag="rh")
            # Weights natural layout: [p, o, n] = W[o*128+p, n]
            wsb = {}
            for name in ("Wir", "Uhr", "Wiz", "Uhz", "Win", "Uhn"):
                wsb[name] = big.tile(
                    [P, NS, D], F16, tag=f"w_{name}", name=f"w_{name}"
                )
            bias = {}
            for name in ("bzt", "brt", "bnt"):
                bt = big.tile([P, NS], F32, tag=name)
                bias[name] = bt

            # ---- inputs on SP queue: (x,h) chunk pairs per o (R-bh0
            # consumption order), then Wiz/Uhz for the ZN phase.
            for o in range(NS):
                for src, dst in ((xt, xT), (ht, hT)):
                    nc.sync.dma_start(
                        out=dst[:, o, :], in_=src.ap()[o * P:(o + 1) * P, :]
                    )
            # All ZN weights also go on the SP queue: it runs no compute, so
            # long transfers never block a drain. The ACT queue carries only
            # the R-critical Wir/Uhr chunks + biases, then is free for
            # sigmoids.
            for name in ("Wiz", "Uhz", "Win", "Uhn"):
                for c in range(2):
                    o0 = c * (NS // 2)
                    nc.sync.dma_start(
                        out=wsb[name][:, o0:o0 + NS // 2, :],
                        in_=wts[name].ap()[o0 * P:(o0 + NS // 2) * P, :]
                        .rearrange("(o p) n -> p o n", p=P),
                    )
            for o in range(NS):
                for name in ("Wir", "Uhr"):
                    nc.scalar.dma_start(
                        out=wsb[name][:, o, :],
                        in_=wts[name].ap()[o * P:(o + 1) * P, :],
                    )
            for name in ("brt", "bzt", "bnt"):
                nc.scalar.dma_start(out=bias[name], in_=bts[name].ap())

            b0 = slice(0, BH)
            b1 = slice(BH, B_CORE)

            # ---- PE warmup: a few matmuls on zeroed scratch while the
            # first DMA chunks are still in flight, so the tensor engine's
            # DVFS is already ramped when the real stream begins.
            scratch = big.tile([P, BH], F16, tag="scratch", name="scratch")
            nc.vector.memset(scratch, 0.0)
            ps_r = [psp.tile([P, BH], F32, tag="mm", name=f"psr{hs}")
                    for hs in range(NS)]
            for _ in range(4):
                nc.tensor.matmul(ps_r[0], scratch[:, 0:P], scratch,
                                 start=True, stop=True)

            # ---- phase R: r = sig(x@Wir + h@Uhr + br); rh = r * hT
            # bh0 pass is o-outer across all 8 hs groups (8 PSUM banks) so
            # each arriving (x,Wir)-chunk o immediately unlocks 8 matmuls --
            # the PE is never blocked behind a group's o=7 chunk while the
            # DMA feed trickles in.
            for o in range(NS):
                for hs in range(NS):
                    nc.tensor.matmul(
                        ps_r[hs], wsb["Wir"][:, o, hs * P:(hs + 1) * P],
                        xT[:, o, b0], start=(o == 0), stop=False,
                    )
                for hs in range(NS):
                    nc.tensor.matmul(
                        ps_r[hs], wsb["Uhr"][:, o, hs * P:(hs + 1) * P],
                        hT[:, o, b0], start=False, stop=(o == NS - 1),
                    )
            for hs in range(NS):
                nc.scalar.activation(
                    rh[:, hs, b0], ps_r[hs], SIG, bias=bias["brt"][:, hs:hs + 1]
                )
                nc.vector.tensor_mul(
                    rh[:, hs, b0], rh[:, hs, b0], hT[:, hs, b0]
                )
            # bh1 pass: everything is resident by now; normal hs-outer groups
            for hs in range(NS):
                cs = slice(hs * P, (hs + 1) * P)
                ps = psp.tile([P, BH], F32, tag="mm")
                for o in range(NS):
                    nc.tensor.matmul(ps, wsb["Wir"][:, o, cs], xT[:, o, b1],
                                     start=(o == 0), stop=False)
                for o in range(NS):
                    nc.tensor.matmul(ps, wsb["Uhr"][:, o, cs], hT[:, o, b1],
                                     start=False, stop=(o == NS - 1))
                nc.scalar.activation(
                    rh[:, hs, b1], ps, SIG, bias=bias["brt"][:, hs:hs + 1]
                )
                nc.vector.tensor_mul(
                    rh[:, hs, b1], rh[:, hs, b1], hT[:, hs, b1]
                )

            # ---- phase ZN + combine
            for hs in range(NS):
                cs = slice(hs * P, (hs + 1) * P)
                ps_z0 = psp.tile([P, BH], F32, tag="mm")
                ps_z1 = psp.tile([P, BH], F32, tag="mm")
                ps_n0 = psp.tile([P, BH], F32, tag="mm")
                ps_n1 = psp.tile([P, BH], F32, tag="mm")
                for o in range(NS):
                    w = wsb["Wiz"][:, o, cs]
                    nc.tensor.matmul(ps_z0, w, xT[:, o, b0],
                                     start=(o == 0), stop=False)
                    nc.tensor.matmul(ps_z1, w, xT[:, o, b1],
                                     start=(o == 0), stop=False)
                for o in range(NS):
                    w = wsb["Uhz"][:, o, cs]
                    nc.tensor.matmul(ps_z0, w, hT[:, o, b0],
                                     start=False, stop=(o == NS - 1))
                    nc.tensor.matmul(ps_z1, w, hT[:, o, b1],
                                     start=False, stop=(o == NS - 1))
                z0 = gt.tile([P, BH], F16, tag="g")
                nc.scalar.activation(z0, ps_z0, SIG, bias=bias["bzt"][:, hs:hs + 1])
                z1 = gt.tile([P, BH], F16, tag="g")
                nc.scalar.activation(z1, ps_z1, SIG, bias=bias["bzt"][:, hs:hs + 1])
                for o in range(NS):
                    w = wsb["Win"][:, o, cs]
                    nc.tensor.matmul(ps_n0, w, xT[:, o, b0],
                                     start=(o == 0), stop=False)
                    nc.tensor.matmul(ps_n1, w, xT[:, o, b1],
                                     start=(o == 0), stop=False)
                for o in range(NS):
                    w = wsb["Uhn"][:, o, cs]
                    nc.tensor.matmul(ps_n0, w, rh[:, o, b0],
                                     start=False, stop=(o == NS - 1))
                    nc.tensor.matmul(ps_n1, w, rh[:, o, b1],
                                     start=False, stop=(o == NS - 1))
                # d = (n - h)*z + h, feature-major, fp16; per-half so the
                # b0 store overlaps the b1 sigmoid/DVE chain. The very last
                # drain (hs=7, b1) is further split in quarters to shorten
                # the post-matmul tail latency.
                for ps_n, bs, zt in ((ps_n0, b0, z0), (ps_n1, b1, z1)):
                    nq = 4 if (hs == NS - 1 and bs is b1) else 1
                    qw = BH // nq
                    for q in range(nq):
                        qs = slice(bs.start + q * qw, bs.start + (q + 1) * qw)
                        pq = slice(q * qw, (q + 1) * qw)
                        nt = gt.tile([P, qw], F16, tag=f"g{nq}")
                        nc.scalar.activation(
                            nt, ps_n[:, pq], SIG, bias=bias["bnt"][:, hs:hs + 1]
                        )
                        d_t = dp.tile([P, qw], F16, tag=f"d{nq}")
                        nc.vector.tensor_sub(d_t, nt, hT[:, hs, qs])
                        nc.vector.tensor_mul(d_t, d_t, zt[:, pq])
                        nc.vector.tensor_add(d_t, d_t, hT[:, hs, qs])
                        # alternate tail stores across both DMA queues so
                        # they dispatch in parallel
                        eng = nc.scalar if (nq > 1 and q % 2 == 1) else nc.sync
                        eng.dma_start(out=out.ap()[cs, qs], in_=d_t)

    nc.compile()
    return nc


def _get_nc():
    if "nc" not in _NC_CACHE:
        _NC_CACHE["nc"] = _build_bass()
    return _NC_CACHE["nc"]


def make_in_maps(inputs):
    f16w = {
        k: np.ascontiguousarray(np.asarray(inputs[k], dtype=np.float16))
        for k in ("Wiz", "Uhz", "Wir", "Uhr", "Win", "Uhn")
    }
    shared = dict(f16w)
    for name, key in (("bzt", "bz"), ("brt", "br"), ("bnt", "bn")):
        shared[name] = np.ascontiguousarray(
            np.asarray(inputs[key], dtype=np.float32).reshape(NS, P).T
        )
    x16 = np.asarray(inputs["x"], dtype=np.float16)
    h16 = np.asarray(inputs["h"], dtype=np.float16)
    in_maps = []
    for c in range(N_CORES):
        sl = slice(c * B_CORE, (c + 1) * B_CORE)
        m = {
            "xt": np.ascontiguousarray(x16[sl].T),
            "ht": np.ascontiguousarray(h16[sl].T),
        }
        m.update(shared)
        in_maps.append(m)
    return in_maps


def kernel(**inputs):
    from concourse.bass_utils import run_bass_kernel_spmd

    nc = _get_nc()
    in_maps = make_in_maps(inputs)
    res = run_bass_kernel_spmd(nc, in_maps, list(range(N_CORES)))
    out = np.concatenate(
        [res.results[c]["out"].T for c in range(N_CORES)], axis=0
    )
    return out.astype(np.float32)


# revision 21
# speedup vs baseline: 1.0596x; 1.0009x over previous
"""Trainium2 Bass kernel for the GRUCell-variant problem.

  z = sigmoid(x@Wiz + h@Uhz + bz)
  r = sigmoid(x@Wir + h@Uhr + br)
  n = sigmoid(x@Win + (r*h)@Uhn + bn)
  out = (1-z)*h + z*n

Full shapes: x,h [8192,1024]; W*,U* [1024,1024]; b* [1024].
Sharding: data-parallel over batch across 8 NeuronCores (1024 rows each);
weights replicated; no collectives.

Design (fp16 compute, fp32 PSUM accumulate, zero device transposes):
  - Host pre-transposes x,h to feature-major [D, B_CORE] fp16 and biases
    to [128,8] fp32; weights are cast to fp16 in natural layout. The
    device output is feature-major [D, B_CORE] fp16; the host transposes
    back and upcasts. All layout shuffling is host-side numpy - the
    device does exclusively matmul + sigmoid + elementwise (median rel
    err ~3.5e-4 vs the fp32 reference).
  - Everything SBUF-resident: xT,hT,rh + 6 weight matrices (fp16 halves
    the footprint so it all fits in ~21MiB of SBUF).
  - 768 matmuls of [128d x 128h stationary] @ [128d x 512b moving] fp16
    (1 cyc/row, ~216ns each): ~166us PE floor, and nothing else runs on
    the PE.
  - DMA queues: SP carries inputs + ZN weights + stores (it runs no
    compute, so a long transfer can never block a PSUM drain); ACT
    carries only the R-critical Wir/Uhr chunks + biases, then is free
    for sigmoids.
  - R-phase batch-half-0 pass is o-outer across all 8 hs PSUM groups so
    each arriving 0.25MiB chunk immediately unlocks 8 matmuls; the PE
    stream measures gapless on HW from first matmul to last.
  - 4 warmup matmuls on zeroed scratch ramp the PE DVFS while the first
    chunks are still in flight; the final drain is split in quarters
    with stores alternated across both DMA queues to shorten the tail.
Measured: 186.7-188.1us HW exec vs 249.0us for the fp32r baseline.
"""

import sys

if "/opt/trn_rl_repo" not in sys.path:
    sys.path.insert(0, "/opt/trn_rl_repo")

import numpy as np

P = 128
B_FULL = 8192
D = 1024  # d_in == d_h == 1024
N_CORES = 8
B_CORE = B_FULL // N_CORES  # 1024
NS = D // P  # 8 strips of 128 along any 1024 dim
BH = 512  # moving free-dim per matmul (one PSUM bank in fp32)
NBH = B_CORE // BH  # 2 batch halves

_NC_CACHE = {}


def _build_bass():
    import concourse.mybir as mybir
    import concourse.tile as tile
    from concourse import bacc

    F16 = mybir.dt.float16
    F32 = mybir.dt.float32
    SIG = mybir.ActivationFunctionType.Sigmoid

    nc = bacc.Bacc(None, target_bir_lowering=False)

    # x,h arrive pre-transposed to feature-major [D, B] fp16
    xt = nc.dram_tensor("xt", [D, B_CORE], F16, kind="ExternalInput")
    ht = nc.dram_tensor("ht", [D, B_CORE], F16, kind="ExternalInput")
    wts = {
        name: nc.dram_tensor(name, [D, D], F16, kind="ExternalInput")
        for name in ("Wiz", "Uhz", "Wir", "Uhr", "Win", "Uhn")
    }
    bts = {
        name: nc.dram_tensor(name, [P, NS], F32, kind="ExternalInput")
        for name in ("bzt", "brt", "bnt")
    }
    # feature-major output; host transposes back
    out = nc.dram_tensor("out", [D, B_CORE], F16, kind="ExternalOutput")

    with tile.TileContext(nc) as tc:
        with (
            tc.tile_pool(name="big", bufs=1) as big,
            tc.tile_pool(name="dp", bufs=4) as dp,
            tc.tile_pool(name="gt", bufs=8) as gt,
            tc.tile_pool(name="ps", bufs=8, space="PSUM") as psp,
        ):
            # Persistent feature-major activations: [p, o, b] = val[o*128+p, b]
            xT = big.tile([P, NS, B_CORE], F16, tag="xT")
            hT = big.tile([P, NS, B_CORE], F16, tag="hT")
            rh = big.tile([P, NS, B_CORE], F16, tag="rh")
            # Weights natural layout: [p, o, n] = W[o*128+p, n]
            wsb = {}
            for name in ("Wir", "Uhr", "Wiz", "Uhz", "Win", "Uhn"):
                wsb[name] = big.tile(
                    [P, NS, D], F16, tag=f"w_{name}", name=f"w_{name}"
                )
            bias = {}
            for name in ("bzt", "brt", "bnt"):
                bt = big.tile([P, NS], F32, tag=name)
                bias[name] = bt

            # ---- inputs on SP queue: (x,h) chunk pairs per o (R-bh0
            # consumption order), then Wiz/Uhz for the ZN phase.
            for o in range(NS):
                for src, dst in ((xt, xT), (ht, hT)):
                    nc.sync.dma_start(
                        out=dst[:, o, :], in_=src.ap()[o * P:(o + 1) * P, :]
                    )
            # All ZN weights also go on the SP queue: it runs no compute, so
            # long transfers never block a drain. The ACT queue carries only
            # the R-critical Wir/Uhr chunks + biases, then is free for
            # sigmoids.
            for name in ("Wiz", "Uhz", "Win", "Uhn"):
                for c in range(2):
                    o0 = c * (NS // 2)
                    nc.sync.dma_start(
                        out=wsb[name][:, o0:o0 + NS // 2, :],
                        in_=wts[name].ap()[o0 * P:(o0 + NS // 2) * P, :]
                        .rearrange("(o p) n -> p o n", p=P),
                    )
            for o in range(NS):
                for name in ("Wir", "Uhr"):
                    nc.scalar.dma_start(
                        out=wsb[name][:, o, :],
                        in_=wts[name].ap()[o * P:(o + 1) * P, :],
                    )
            for name in ("brt", "bzt", "bnt"):
                nc.scalar.dma_start(out=bias[name], in_=bts[name].ap())

            b0 = slice(0, BH)
            b1 = slice(BH, B_CORE)

            # ---- PE warmup: matmuls on zeroed scratch while the first DMA
            # chunks are still in flight, so the tensor engine's DVFS is
            # already ramped when the real stream begins. The memset runs on
            # GpSimd, whose preamble finishes ~1.5us before Vector's, so the
            # warmup covers the whole 3us ramp window before real data lands.
            scratch = big.tile([P, BH], F16, tag="scratch", name="scratch")
            nc.gpsimd.memset(scratch, 0.0)
            ps_r = [psp.tile([P, BH], F32, tag="mm", name=f"psr{hs}")
                    for hs in range(NS)]
            for _ in range(6):
                nc.tensor.matmul(ps_r[0], scratch[:, 0:P], scratch,
                                 start=True, stop=True)

            # ---- phase R: r = sig(x@Wir + h@Uhr + br); rh = r * hT
            # bh0 pass is o-outer across all 8 hs groups (8 PSUM banks) so
            # each arriving (x,Wir)-chunk o immediately unlocks 8 matmuls --
            # the PE is never blocked behind a group's o=7 chunk while the
            # DMA feed trickles in.
            for o in range(NS):
                for hs in range(NS):
                    nc.tensor.matmul(
                        ps_r[hs], wsb["Wir"][:, o, hs * P:(hs + 1) * P],
                        xT[:, o, b0], start=(o == 0), stop=False,
                    )
                for hs in range(NS):
                    nc.tensor.matmul(
                        ps_r[hs], wsb["Uhr"][:, o, hs * P:(hs + 1) * P],
                        hT[:, o, b0], start=False, stop=(o == NS - 1),
                    )
            for hs in range(NS):
                nc.scalar.activation(
                    rh[:, hs, b0], ps_r[hs], SIG, bias=bias["brt"][:, hs:hs + 1]
                )
                nc.vector.tensor_mul(
                    rh[:, hs, b0], rh[:, hs, b0], hT[:, hs, b0]
                )
            # bh1 pass: everything is resident by now; normal hs-outer groups
            for hs in range(NS):
                cs = slice(hs * P, (hs + 1) * P)
                ps = psp.tile([P, BH], F32, tag="mm")
                for o in range(NS):
                    nc.tensor.matmul(ps, wsb["Wir"][:, o, cs], xT[:, o, b1],
                                     start=(o == 0), stop=False)
                for o in range(NS):
                    nc.tensor.matmul(ps, wsb["Uhr"][:, o, cs], hT[:, o, b1],
                                     start=False, stop=(o == NS - 1))
                nc.scalar.activation(
                    rh[:, hs, b1], ps, SIG, bias=bias["brt"][:, hs:hs + 1]
                )
                nc.vector.tensor_mul(
                    rh[:, hs, b1], rh[:, hs, b1], hT[:, hs, b1]
                )

            # ---- phase ZN + combine
            for hs in range(NS):
                cs = slice(hs * P, (hs + 1) * P)
                ps_z0 = psp.tile([P, BH], F32, tag="mm")
                ps_z1 = psp.tile([P, BH], F32, tag="mm")
                ps_n0 = psp.tile([P, BH], F32, tag="mm")
                ps_n1 = psp.tile([P, BH], F32, tag="mm")
                for o in range(NS):
                    w = wsb["Wiz"][:, o, cs]
                    nc.tensor.matmul(ps_z0, w, xT[:, o, b0],
                                     start=(o == 0), stop=False)
                    nc.tensor.matmul(ps_z1, w, xT[:, o, b1],
                                     start=(o == 0), stop=False)
                for o in range(NS):
                    w = wsb["Uhz"][:, o, cs]
                    nc.tensor.matmul(ps_z0, w, hT[:, o, b0],
                                     start=False, stop=(o == NS - 1))
                    nc.tensor.matmul(ps_z1, w, hT[:, o, b1],
                                     start=False, stop=(o == NS - 1))
                z0 = gt.tile([P, BH], F16, tag="g")
                nc.scalar.activation(z0, ps_z0, SIG, bias=bias["bzt"][:, hs:hs + 1])
                z1 = gt.tile([P, BH], F16, tag="g")
                nc.scalar.activation(z1, ps_z1, SIG, bias=bias["bzt"][:, hs:hs + 1])
                def drain(ps_ap, bs, zt, pq0, nq, qw):
                    """sigmoid + combine + store for a [*, nq*qw] slice of a
                    gate psum; pq0 is the column offset inside the psum/z
                    tiles, bs.start+pq0 the batch offset."""
                    for q in range(nq):
                        p0 = pq0 + q * qw
                        qs = slice(bs.start + p0, bs.start + p0 + qw)
                        pq = slice(p0, p0 + qw)
                        nt = gt.tile([P, qw], F16, tag=f"g{qw}", name="nt")
                        nc.scalar.activation(
                            nt, ps_ap[:, pq], SIG,
                            bias=bias["bnt"][:, hs:hs + 1]
                        )
                        d_t = dp.tile([P, qw], F16, tag=f"d{qw}", name="d_t")
                        nc.vector.tensor_sub(d_t, nt, hT[:, hs, qs])
                        nc.vector.tensor_mul(d_t, d_t, zt[:, pq])
                        nc.vector.tensor_add(d_t, d_t, hT[:, hs, qs])
                        # alternate tail stores across both DMA queues so
                        # they dispatch in parallel
                        eng = nc.scalar if (nq > 1 and q % 2 == 1) else nc.sync
                        eng.dma_start(out=out.ap()[cs, qs], in_=d_t)

                if hs < NS - 1:
                    for o in range(NS):
                        w = wsb["Win"][:, o, cs]
                        nc.tensor.matmul(ps_n0, w, xT[:, o, b0],
                                         start=(o == 0), stop=False)
                        nc.tensor.matmul(ps_n1, w, xT[:, o, b1],
                                         start=(o == 0), stop=False)
                    for o in range(NS):
                        w = wsb["Uhn"][:, o, cs]
                        nc.tensor.matmul(ps_n0, w, rh[:, o, b0],
                                         start=False, stop=(o == NS - 1))
                        nc.tensor.matmul(ps_n1, w, rh[:, o, b1],
                                         start=False, stop=(o == NS - 1))
                    drain(ps_n0, b0, z0, 0, 1, BH)
                    drain(ps_n1, b1, z1, 0, 1, BH)
                else:
                    # last hs: three sequential groups (b0 full, then the
                    # two b1 halves on disjoint psum column ranges) so each
                    # drain overlaps the next group's matmuls and the final
                    # post-matmul tail covers only 256 columns.
                    for o in range(NS):
                        nc.tensor.matmul(ps_n0, wsb["Win"][:, o, cs],
                                         xT[:, o, b0],
                                         start=(o == 0), stop=False)
                    for o in range(NS):
                        nc.tensor.matmul(ps_n0, wsb["Uhn"][:, o, cs],
                                         rh[:, o, b0],
                                         start=False, stop=(o == NS - 1))
                    drain(ps_n0, b0, z0, 0, 2, BH // 2)
                    for half in range(2):
                        hw_ = BH // 2
                        bq = slice(BH + half * hw_, BH + (half + 1) * hw_)
                        pq = slice(half * hw_, (half + 1) * hw_)
                        for o in range(NS):
                            nc.tensor.matmul(
                                ps_n1[:, pq], wsb["Win"][:, o, cs],
                                xT[:, o, bq], start=(o == 0), stop=False,
                            )
                        for o in range(NS):
                            nc.tensor.matmul(
                                ps_n1[:, pq], wsb["Uhn"][:, o, cs],
                                rh[:, o, bq],
                                start=False, stop=(o == NS - 1),
                            )
                        drain(ps_n1, b1, z1, half * hw_, 2, hw_ // 2)

    nc.compile()
    return nc


def _get_nc():
    if "nc" not in _NC_CACHE:
        _NC_CACHE["nc"] = _build_bass()
    return _NC_CACHE["nc"]


def make_in_maps(inputs):
    f16w = {
        k: np.ascontiguousarray(np.asarray(inputs[k], dtype=np.float16))
        for k in ("Wiz", "Uhz", "Wir", "Uhr", "Win", "Uhn")
    }
    shared = dict(f16w)
    for name, key in (("bzt", "bz"), ("brt", "br"), ("bnt", "bn")):
        shared[name] = np.ascontiguousarray(
            np.asarray(inputs[key], dtype=np.float32).reshape(NS, P).T
        )
    x16 = np.asarray(inputs["x"], dtype=np.float16)
    h16 = np.asarray(inputs["h"], dtype=np.float16)
    in_maps = []
    for c in range(N_CORES):
        sl = slice(c * B_CORE, (c + 1) * B_CORE)
        m = {
            "xt": np.ascontiguousarray(x16[sl].T),
            "ht": np.ascontiguousarray(h16[sl].T),
        }
        m.update(shared)
        in_maps.append(m)
    return in_maps


def kernel(**inputs):
    from concourse.bass_utils import run_bass_kernel_spmd

    nc = _get_nc()
    in_maps = make_in_maps(inputs)
    res = run_bass_kernel_spmd(nc, in_maps, list(range(N_CORES)))
    out = np.concatenate(
        [res.results[c]["out"].T for c in range(N_CORES)], axis=0
    )
    return out.astype(np.float32)
